# revision 29
# baseline (speedup 1.0000x reference)
"""Trainium2 Bass kernel for nn_AdaptiveMultiBoxLoss (SSD multibox distillation loss).

Data-parallel over the batch dim across 8 NeuronCores; host sums the 8x16
partial columns and performs the final division by N.

v3 design:
  - host casts conf/loc inputs to bf16: HBM traffic 49 MB -> 26 MB per core
  - PE does the conf[p, ct_p] one-hot trace matmul in bf16
  - sumexp per prior: fold level 1 on DVE (bf16 2x), levels 2-6 on the
    otherwise-idle Pool engine, f32 finals on DVE
  - all Ln batched at the end; smooth-L1 in the exact
    0.5*min(u,1)^2 + (u - min(u,1)) form (masked elements contribute 0)
  - hard-negative top-k: lcm is re-partitioned via an SBUF->SBUF DMA into a
    (row, replica) chunk layout so each partition's threshold is a
    per-partition tensor_scalar operand (4x DVE mode, fused count accum);
    the exact pass is a single max-accum per tensor:
    topk = sum(max(v, lo)) + (k - 8832) * lo
  - prior tiling: partition p holds priors [69p, 69p+69); partition 126 is
    end-aligned (priors 8663..8731, first 31 slots masked as duplicates of
    p125's tail); partition 127 is zero-filled by a small const DMA
"""

import os
import sys

sys.path.insert(0, "/opt/trn_rl_repo")

from contextlib import ExitStack

import ml_dtypes
import numpy as np

import concourse.bass as bass
import concourse.bacc as bacc
import concourse.mybir as mybir
import concourse.tile as tile
from concourse.bass_utils import run_bass_kernel_spmd

F32 = mybir.dt.float32
BF16 = mybir.dt.bfloat16
I32 = mybir.dt.int32
ALU = mybir.AluOpType
ACT = mybir.ActivationFunctionType

# ---- problem geometry (hardcoded) ----
B, P, C = 64, 8732, 81
NCORES = 8
R = B // NCORES            # 8 batch rows per core
NT = 69                    # priors per partition per row (126*69+38 = 8732)
P_FULL = 126 * NT          # 8694 priors on partitions 0..125
TAIL = P - P_FULL          # 38 real priors on partition 126
TAIL_OFF = NT - TAIL       # 31 duplicate slots at the start of p126
RC = R * NT                # 552 row-tiled columns
NREP = 16                  # search chunk replicas per row (16*552 = 8832 slots)
NSLOT = 128 * NT // NREP   # 552 lcm slots per chunk partition
LTT = 546                  # flat loc tiling: 128 * 546 >= R * P
LF = LTT * 4               # 2184
PADN = 128 * LTT - R * P
NPART = 16
NITER = 7                  # binary search iterations (thresholds stay
HI_INIT = 16.0             # bf16-exact: dyadics with <= 8 mantissa bits)

# partials columns
(COL_BT, COL_BS, COL_AT, COL_AS, COL_CT, COL_CS, COL_DT, COL_DS,
 COL_LT, COL_LS, COL_TKT, COL_TKS, COL_NP) = range(13)

STAGE = int(os.environ.get("K_STAGE", "9"))
# fold levels handled by the Pool engine (per-level start column widths)
POOL_LVLS = set(
    int(x) for x in os.environ.get("K_POOL_LVLS", "2,3,4,5,6").split(",") if x)
# tensor_scalar fused accumulate (TensorScalarPtrReduce) for search sums
TS_ACCUM = os.environ.get("K_TS_ACCUM", "0") == "1"


def build_nc():
    nc = bacc.Bacc("TRN2", target_bir_lowering=False, debug=False,
                   num_devices=NCORES)

    conf_T = nc.declare_dram_parameter("conf_T", [R, P, C], BF16, isOutput=False)
    conf_S = nc.declare_dram_parameter("conf_S", [R, P, C], BF16, isOutput=False)
    loc_T = nc.declare_dram_parameter("loc_T", [128 * LTT, 4], BF16, isOutput=False)
    loc_S = nc.declare_dram_parameter("loc_S", [128 * LTT, 4], BF16, isOutput=False)
    loc_t = nc.declare_dram_parameter("loc_t", [128 * LTT, 4], BF16, isOutput=False)
    ctbf_p = nc.declare_dram_parameter("ctbf", [128, RC], BF16, isOutput=False)
    ctfl_p = nc.declare_dram_parameter("ctfl", [128, LTT], BF16, isOutput=False)
    eye81_p = nc.declare_dram_parameter("eye81", [81, 81], BF16, isOutput=False)
    g8_p = nc.declare_dram_parameter("g8", [128, 8], F32, isOutput=False)
    g8t_p = nc.declare_dram_parameter("g8t", [8, 128], F32, isOutput=False)
    ones_p = nc.declare_dram_parameter("ones128", [128, 1], F32, isOutput=False)
    zpad_p = nc.declare_dram_parameter("zpad", [1, NT * C], BF16, isOutput=False)
    scr_p = nc.declare_dram_parameter("scr", [2, 128 * NSLOT], BF16, isOutput=False)
    out_p = nc.declare_dram_parameter("out", [1, NPART], F32, isOutput=True)

    with tile.TileContext(nc) as tc, ExitStack() as ctx:
        cpool = ctx.enter_context(tc.tile_pool(name="consts", bufs=1))
        pers = ctx.enter_context(tc.tile_pool(name="pers", bufs=1))
        small = ctx.enter_context(tc.tile_pool(name="small", bufs=1))
        pool_c = ctx.enter_context(tc.tile_pool(name="conf", bufs=2))
        pool_e = ctx.enter_context(tc.tile_pool(name="expx", bufs=2))
        pool_q = ctx.enter_context(tc.tile_pool(name="eq", bufs=2))
        pool_f = ctx.enter_context(tc.tile_pool(name="fold", bufs=2))
        psum = ctx.enter_context(tc.tile_pool(name="ps", bufs=4, space="PSUM"))
        pstr = ctx.enter_context(tc.tile_pool(name="tr", bufs=1, space="PSUM"))

        # ---- persistent tensors ----
        ctbf = pers.tile([128, RC], BF16)
        posf = pers.tile([128, RC], BF16)
        valid = pers.tile([128, RC], BF16)
        ominus = pers.tile([128, RC], BF16)
        sumexp2 = pers.tile([128, 2, RC], F32)      # becomes lse2 in place
        conf02 = pers.tile([128, 2, RC], F32)
        lcm2 = {xi: pers.tile([128, RC], BF16, name=f"lcm{xi}")
                for xi in range(2)}
        lcmc = {xi: pers.tile([128, NSLOT], BF16, name=f"lcmc{xi}")
                for xi in range(2)}                 # chunked search layout
        sjc = {xi: pers.tile([128, NSLOT], BF16, name=f"sjc{xi}")
               for xi in range(2)}
        sjunk2 = pers.tile([128, 2, RC], F32)
        partials = pers.tile([128, NPART], F32)

        ctflb = pers.tile([128, LTT], BF16)
        posml = pers.tile([128, LTT], BF16)
        posml4 = pers.tile([128, LF], BF16)
        locsb = {n: pers.tile([128, LF], BF16, name=f"loc{n}")
                 for n in ("T", "S", "t")}
        lwu = pers.tile([128, LF], BF16)
        lws = pers.tile([128, LF], BF16)
        lwd = pers.tile([128, LF], BF16)

        # ---- constants ----
        eye81 = cpool.tile([81, 81], BF16)
        g8 = cpool.tile([128, 8], F32)
        g8t = cpool.tile([8, 128], F32)
        ones128 = cpool.tile([128, 1], F32)
        iota81 = cpool.tile([128, 81], BF16)

        def emit_consts():
            nc.sync.dma_start(out=eye81[:, :], in_=eye81_p.ap())
            nc.sync.dma_start(out=g8[:, :], in_=g8_p.ap())
            nc.sync.dma_start(out=g8t[:, :], in_=g8t_p.ap())
            nc.sync.dma_start(out=ones128[:, :], in_=ones_p.ap())
            nc.gpsimd.iota(iota81[:, :], pattern=[[1, 81]], base=0,
                           channel_multiplier=0,
                           allow_small_or_imprecise_dtypes=True)
            nc.gpsimd.memset(partials[:, :], 0.0)
            nc.sync.dma_start(out=ctbf[:, :], in_=ctbf_p.ap())
            nc.vector.tensor_scalar(out=posf[:, :], in0=ctbf[:, :],
                                    scalar1=0.5, scalar2=None, op0=ALU.is_gt)
            nc.vector.tensor_scalar(out=valid[:, :], in0=ctbf[:, :],
                                    scalar1=-0.5, scalar2=None, op0=ALU.is_gt)
            nc.vector.tensor_tensor(out=ominus[:, :], in0=valid[:, :],
                                    in1=posf[:, :], op=ALU.subtract)

        def emit_loc_dmas():
            nc.sync.dma_start(out=ctflb[:, :], in_=ctfl_p.ap())
            for name, param in (("T", loc_T), ("S", loc_S), ("t", loc_t)):
                nc.sync.dma_start(
                    out=locsb[name][:, :],
                    in_=param.ap().rearrange("(p j) f -> p (j f)", j=LTT))

        def emit_loc_chain():
            nc.vector.tensor_scalar(out=posml[:, :], in0=ctflb[:, :],
                                    scalar1=0.5, scalar2=None, op0=ALU.is_gt)
            # posml4[p, 4j+f] = posml[p, j]
            nc.vector.tensor_copy(
                out=posml4[:, :].rearrange("p (j f) -> p j f", f=4),
                in_=posml[:, :].unsqueeze(2).broadcast_to((128, LTT, 4)))
            for x, col in (("T", COL_LT), ("S", COL_LS)):
                nc.vector.tensor_tensor(out=lwd[:, :], in0=locsb[x][:, :],
                                        in1=locsb["t"][:, :], op=ALU.subtract)
                nc.vector.tensor_tensor(out=lwu[:, :], in0=lwd[:, :],
                                        in1=posml4[:, :], op=ALU.mult)
                nc.scalar.activation(out=lwu[:, :], in_=lwu[:, :], func=ACT.Abs)
                nc.vector.tensor_scalar(out=lws[:, :], in0=lwu[:, :],
                                        scalar1=1.0, scalar2=None, op0=ALU.min)
                # lwd = 0.5 * s^2
                nc.scalar.activation(out=lwd[:, :], in_=lws[:, :],
                                     func=ACT.Square,
                                     scale=float(1.0 / np.sqrt(2.0)))
                nc.vector.tensor_tensor(out=lwu[:, :], in0=lwu[:, :],
                                        in1=lws[:, :], op=ALU.subtract)
                nc.vector.tensor_tensor(out=lwd[:, :], in0=lwd[:, :],
                                        in1=lwu[:, :], op=ALU.add)
                nc.vector.tensor_reduce(out=partials[:, col:col + 1],
                                        in_=lwd[:, :],
                                        axis=mybir.AxisListType.X, op=ALU.add)

        # ---- conf streaming loop ----
        pstr2 = pstr.tile([81, 2, C], F32)
        nmm = 0
        total_mm = R * NT

        for r in range(R):
            rc = r * NT
            ctile = pool_c.tile([128, 2, NT, C], BF16, name="ctile")
            for xi, (x, param) in enumerate((("T", conf_T), ("S", conf_S))):
                rowv = param.ap()[r, :, :]
                nc.sync.dma_start(
                    out=ctile[0:126, xi, :, :],
                    in_=rowv[0:P_FULL, :].rearrange("(p t) c -> p t c", t=NT))
                nc.sync.dma_start(
                    out=ctile[126:127, xi, :, :],
                    in_=rowv[P - NT:P, :].unsqueeze(0))
                if r < 2:
                    # p127 is never written by the row DMAs: fill with zeros
                    # once per buffer slot so exp() stays finite
                    nc.sync.dma_start(
                        out=ctile[127:128, xi, :, :],
                        in_=zpad_p.ap().rearrange("o (t c) -> o t c", c=C))
            if r == 0:
                emit_consts()
            if r == 1:
                emit_loc_dmas()

            ex = pool_e.tile([128, 2, NT, C], BF16, name="ex")
            for xi in range(2):
                nc.scalar.activation(out=ex[:, xi, :, :],
                                     in_=ctile[:, xi, :, :], func=ACT.Exp)

            # one-hot eq (broadcast operands: no DVE fast path exists)
            eq = pool_q.tile([128, NT, C], BF16, name="eq")
            if STAGE >= 3:
                nc.vector.tensor_tensor(
                    out=eq[:, :, :],
                    in0=iota81[:, :].unsqueeze(1).broadcast_to((128, NT, C)),
                    in1=ctbf[:, rc:rc + NT].unsqueeze(2).broadcast_to(
                        (128, NT, C)),
                    op=ALU.is_equal)

                # B trace: psum[m, xi, c] += sum_p eq[p,t,m] * conf[p,xi,t,c]
                for t in range(NT):
                    nc.tensor.matmul(
                        pstr2[:, :, :],
                        lhsT=eq[:, t, :],
                        rhs=ctile[:, :, t, :],
                        start=(nmm == 0), stop=(nmm == total_mm - 1))
                    nmm += 1

            # sumexp fold-tree: level 1 on DVE (bf16 2x), rest on Pool
            if STAGE >= 4:
                fold = pool_f.tile([128, 2, NT, 40], BF16, name="fold")
                for xi in range(2):
                    f = fold[:, xi, :, :]
                    e = ex[:, xi, :, :]
                    lvls = [(1, 40, e), (2, 20, None), (3, 10, None),
                            (4, 5, None), (5, 2, None), (6, 1, None)]
                    for lvl, w, src in lvls:
                        eng = nc.gpsimd if lvl in POOL_LVLS else nc.vector
                        if src is not None:
                            eng.tensor_tensor(out=f[:, :, 0:w],
                                              in0=src[:, :, 0:40],
                                              in1=src[:, :, 40:80], op=ALU.add)
                        else:
                            eng.tensor_tensor(out=f[:, :, 0:w],
                                              in0=f[:, :, 0:w],
                                              in1=f[:, :, w:2 * w], op=ALU.add)
                    se = sumexp2[:, xi, rc:rc + NT]
                    nc.vector.tensor_tensor(out=se, in0=f[:, :, 0],
                                            in1=f[:, :, 4], op=ALU.add)
                    nc.vector.tensor_tensor(out=se, in0=se,
                                            in1=e[:, :, 80], op=ALU.add)
                    nc.vector.tensor_copy(out=conf02[:, xi, rc:rc + NT],
                                          in_=ctile[:, xi, :, 0])

            if r == 3:
                emit_loc_chain()

        # num_pos per row -> k (only needed by the tail search)
        npp = small.tile([128, 8], F32)
        nc.vector.tensor_reduce(out=npp[:, :],
                                in_=posf[:, :].rearrange("p (r t) -> p r t", r=R),
                                axis=mybir.AxisListType.X, op=ALU.add)
        ps_np = psum.tile([8, 1], F32, tag="ps")
        nc.tensor.matmul(ps_np[:, :], lhsT=npp[:, :], rhs=ones128[:, :],
                         start=True, stop=True)
        np8 = small.tile([8, 1], F32)
        nc.vector.tensor_copy(out=np8[:, :], in_=ps_np[:, :])
        k82 = small.tile([8, 2], F32)
        nc.vector.tensor_scalar(out=k82[:, 0:1], in0=np8[:, :], scalar1=3.0,
                                scalar2=float(P - 1), op0=ALU.mult, op1=ALU.min)
        nc.vector.tensor_copy(out=k82[:, 1:2], in_=k82[:, 0:1])
        nc.vector.tensor_copy(out=partials[0:8, COL_NP:COL_NP + 1],
                              in_=np8[:, :])

        # ---- tail: lse, lcm, epilogue sums ----
        if STAGE >= 5:
            # lse in place of sumexp (one Ln, one table load)
            nc.scalar.activation(out=sumexp2[:, :, :], in_=sumexp2[:, :, :],
                                 func=ACT.Ln)
            lse2 = sumexp2
            for xi in range(2):
                nc.vector.tensor_tensor(out=lcm2[xi][:, :],
                                        in0=lse2[:, xi, :],
                                        in1=conf02[:, xi, :], op=ALU.subtract)
                nc.vector.tensor_tensor(out=lcm2[xi][:, :],
                                        in0=lcm2[xi][:, :], in1=ominus[:, :],
                                        op=ALU.mult)
                # re-partition lcm into the chunk layout via a DRAM bounce:
                # chunk partition q = 16r + k holds source partitions
                # 8k..8k+7 of row r.  (A direct SBUF->SBUF partition-split
                # rearrange lowers to out-of-bounds free-dim strides.)
                nc.sync.dma_start(
                    out=scr_p.ap()[xi, :].rearrange(
                        "(r k i t) -> k i r t", r=R, k=NREP, i=8),
                    in_=lcm2[xi][:, :].rearrange(
                        "(k i) (r t) -> (k i) r t", k=NREP, r=R))
                nc.sync.dma_start(
                    out=lcmc[xi][:, :],
                    in_=scr_p.ap()[xi, :].rearrange("(q j) -> q j", q=128))

            # A = sum(lse*posf), C = sum(conf0*valid), D = sum(conf0*posf)
            for src, mask, (c0, c1) in (
                    (lse2, posf, (COL_AT, COL_AS)),
                    (conf02, valid, (COL_CT, COL_CS)),
                    (conf02, posf, (COL_DT, COL_DS))):
                nc.vector.tensor_tensor(
                    out=sjunk2[:, :, :], in0=src[:, :, :],
                    in1=mask[:, :].unsqueeze(1).broadcast_to((128, 2, RC)),
                    op=ALU.mult)
                assert c1 == c0 + 1
                nc.vector.tensor_reduce(out=partials[:, c0:c1 + 1],
                                        in_=sjunk2[:, :, :],
                                        axis=mybir.AxisListType.X, op=ALU.add)

            # B extraction from the PSUM trace
            nc.vector.tensor_tensor(
                out=sjunk2[0:81, :, 0:81], in0=pstr2[:, :, :],
                in1=eye81[:, :].unsqueeze(1).broadcast_to((81, 2, 81)),
                op=ALU.mult)
            nc.vector.tensor_reduce(out=partials[0:81, COL_BT:COL_BS + 1],
                                    in_=sjunk2[0:81, :, 0:81],
                                    axis=mybir.AxisListType.X, op=ALU.add)

        # ---- chunked binary search for per-row top-k thresholds ----
        if STAGE >= 6:
            lo82 = small.tile([8, 2], F32)
            hi82 = small.tile([8, 2], F32)
            tm82 = small.tile([8, 2], F32)
            s82 = small.tile([8, 2], F32)
            ge82 = small.tile([8, 2], I32)
            gei82 = small.tile([8, 2], I32)
            trep2 = small.tile([128, 2], F32)
            cnt2 = small.tile([128, 2], F32)
            sm2 = small.tile([128, 2], F32)
            tk82 = small.tile([8, 2], F32)

            def bcast82(vec82):
                psT = psum.tile([128, 2], F32, name="psT", tag="ps")
                nc.tensor.matmul(psT[:, :], lhsT=g8t[:, :], rhs=vec82[:, :],
                                 start=True, stop=True)
                nc.vector.tensor_copy(out=trep2[:, :], in_=psT[:, :])

            nc.gpsimd.memset(lo82[:, :], 0.0)
            nc.gpsimd.memset(hi82[:, :], HI_INIT)
            for it in range(NITER):
                nc.vector.tensor_tensor(out=tm82[:, :], in0=lo82[:, :],
                                        in1=hi82[:, :], op=ALU.add)
                nc.vector.tensor_scalar(out=tm82[:, :], in0=tm82[:, :],
                                        scalar1=0.5, scalar2=None, op0=ALU.mult)
                bcast82(tm82)
                for xi in range(2):
                    if TS_ACCUM:
                        nc.vector.tensor_scalar(
                            out=sjc[xi][:, :], in0=lcmc[xi][:, :],
                            scalar1=trep2[:, xi:xi + 1], scalar2=0.0,
                            op0=ALU.is_gt, op1=ALU.add,
                            accum_out=cnt2[:, xi:xi + 1])
                    else:
                        nc.vector.tensor_scalar(
                            out=sjc[xi][:, :], in0=lcmc[xi][:, :],
                            scalar1=trep2[:, xi:xi + 1], scalar2=None,
                            op0=ALU.is_gt)
                        nc.vector.tensor_reduce(
                            out=cnt2[:, xi:xi + 1], in_=sjc[xi][:, :],
                            axis=mybir.AxisListType.X, op=ALU.add)
                psN = psum.tile([8, 2], F32, name="psN", tag="ps")
                nc.tensor.matmul(psN[:, :], lhsT=g8[:, :], rhs=cnt2[:, :],
                                 start=True, stop=True)
                nc.vector.tensor_copy(out=s82[:, :], in_=psN[:, :])
                nc.vector.tensor_tensor(out=ge82[:, :], in0=s82[:, :],
                                        in1=k82[:, :], op=ALU.is_ge)
                nc.vector.copy_predicated(out=lo82[:, :], mask=ge82[:, :],
                                          data=tm82[:, :])
                nc.vector.tensor_scalar(out=gei82[:, :], in0=ge82[:, :],
                                        scalar1=1, scalar2=None,
                                        op0=ALU.bitwise_xor)
                nc.vector.copy_predicated(out=hi82[:, :], mask=gei82[:, :],
                                          data=tm82[:, :])

            # exact pass: topk = sum(max(v, lo)) + (k - 8832) * lo
            bcast82(lo82)
            for xi in range(2):
                if TS_ACCUM:
                    nc.vector.tensor_scalar(
                        out=sjc[xi][:, :], in0=lcmc[xi][:, :],
                        scalar1=trep2[:, xi:xi + 1], scalar2=0.0,
                        op0=ALU.max, op1=ALU.add,
                        accum_out=sm2[:, xi:xi + 1])
                else:
                    nc.vector.tensor_scalar(
                        out=sjc[xi][:, :], in0=lcmc[xi][:, :],
                        scalar1=trep2[:, xi:xi + 1], scalar2=None,
                        op0=ALU.max)
                    nc.vector.tensor_reduce(
                        out=sm2[:, xi:xi + 1], in_=sjc[xi][:, :],
                        axis=mybir.AxisListType.X, op=ALU.add)
            psE = psum.tile([8, 2], F32, name="psE", tag="ps")
            nc.tensor.matmul(psE[:, :], lhsT=g8[:, :], rhs=sm2[:, :],
                             start=True, stop=True)
            nc.vector.tensor_copy(out=tk82[:, :], in_=psE[:, :])
            nc.vector.tensor_scalar(out=s82[:, :], in0=k82[:, :],
                                    scalar1=float(NREP * NSLOT), scalar2=None,
                                    op0=ALU.subtract)
            nc.vector.tensor_tensor(out=s82[:, :], in0=s82[:, :],
                                    in1=lo82[:, :], op=ALU.mult)
            nc.vector.tensor_tensor(out=tk82[:, :], in0=tk82[:, :],
                                    in1=s82[:, :], op=ALU.add)
            nc.vector.tensor_copy(out=partials[0:8, COL_TKT:COL_TKT + 1],
                                  in_=tk82[:, 0:1])
            nc.vector.tensor_copy(out=partials[0:8, COL_TKS:COL_TKS + 1],
                                  in_=tk82[:, 1:2])

        # ---- final partition reduce of partials -> out ----
        psF = psum.tile([1, NPART], F32, name="psF", tag="ps")
        nc.tensor.matmul(psF[:, :], lhsT=ones128[:, :], rhs=partials[:, :],
                         start=True, stop=True)
        fin = small.tile([1, NPART], F32)
        nc.vector.tensor_copy(out=fin[:, :], in_=psF[:, :])
        nc.sync.dma_start(out=out_p.ap(), in_=fin[:, :])
    nc.finalize()
    return nc


_NC_CACHE = None


def _get_nc():
    global _NC_CACHE
    if _NC_CACHE is None:
        _NC_CACHE = build_nc()
    return _NC_CACHE


def _host_consts():
    eye81 = np.eye(81, dtype=ml_dtypes.bfloat16)
    g8 = np.zeros((128, 8), np.float32)
    for p in range(128):
        g8[p, p // NREP] = 1.0
    g8t = np.ascontiguousarray(g8.T)
    ones128 = np.ones((128, 1), np.float32)
    zpad = np.zeros((1, NT * C), ml_dtypes.bfloat16)
    return eye81, g8, g8t, ones128, zpad


def _ct_row_tiled(ct_rows: np.ndarray) -> np.ndarray:
    """[R, P] int -> [128, R*NT] bf16 row-tiled, pads/duplicates = -1."""
    out = np.full((128, RC), -1.0, np.float32)
    for r in range(R):
        out[0:126, r * NT:(r + 1) * NT] = ct_rows[r, 0:P_FULL].reshape(126, NT)
        out[126, r * NT + TAIL_OFF:(r + 1) * NT] = ct_rows[r, P_FULL:P]
    return out.astype(ml_dtypes.bfloat16)


def _ct_flat(ct_rows: np.ndarray) -> np.ndarray:
    flat = np.full(128 * LTT, -1.0, np.float32)
    flat[:R * P] = ct_rows.reshape(-1)
    return flat.reshape(128, LTT).astype(ml_dtypes.bfloat16)


def _build_in_maps(inputs):
    conf_T = np.asarray(inputs["conf_dataT"], np.float32)
    conf_S = np.asarray(inputs["conf_dataS"], np.float32)
    loc_T = np.asarray(inputs["loc_dataT"], np.float32)
    loc_S = np.asarray(inputs["loc_dataS"], np.float32)
    loc_t = np.asarray(inputs["loc_t"], np.float32)
    ct = np.asarray(inputs["conf_t"], np.int32)

    def _padloc(a):
        flat = a.reshape(R * P, 4)
        return np.ascontiguousarray(
            np.pad(flat, ((0, PADN), (0, 0)))).astype(ml_dtypes.bfloat16)

    eye81, g8, g8t, ones128, zpad = _host_consts()
    in_maps = []
    for d in range(NCORES):
        sl = slice(d * R, (d + 1) * R)
        ctsl = ct[sl]
        in_maps.append({
            "conf_T": np.ascontiguousarray(conf_T[sl]).astype(ml_dtypes.bfloat16),
            "conf_S": np.ascontiguousarray(conf_S[sl]).astype(ml_dtypes.bfloat16),
            "loc_T": _padloc(loc_T[sl]), "loc_S": _padloc(loc_S[sl]),
            "loc_t": _padloc(loc_t[sl]),
            "ctbf": _ct_row_tiled(ctsl),
            "ctfl": _ct_flat(ctsl),
            "eye81": eye81, "g8": g8, "g8t": g8t,
            "ones128": ones128, "zpad": zpad,
            "scr": np.zeros((2, 128 * NSLOT), ml_dtypes.bfloat16),
        })
    return in_maps


def _combine(parts):
    S = parts.astype(np.float64).sum(axis=0)
    loss_cT = S[COL_AT] - S[COL_BT] + S[COL_CT] - S[COL_DT] + S[COL_TKT]
    loss_cS = S[COL_AS] - S[COL_BS] + S[COL_CS] - S[COL_DS] + S[COL_TKS]
    N = S[COL_NP]
    return np.array([S[COL_LT] / N, loss_cT / N, S[COL_LS] / N, loss_cS / N],
                    np.float32)


def run_on_hw(inputs, trace=False, **kw):
    nc = _get_nc()
    in_maps = _build_in_maps(inputs)
    res = run_bass_kernel_spmd(nc, in_maps, core_ids=list(range(NCORES)),
                               trace=trace, **kw)
    parts = np.stack([np.asarray(r["out"]).reshape(NPART) for r in res.results])
    return _combine(parts), res


def kernel(**inputs) -> np.ndarray:
    out, _ = run_on_hw(inputs, trace=False)
    return out


# revision 36
# speedup vs baseline: 1.3588x; 1.3588x over previous
"""Trainium2 Bass kernel for nn_AdaptiveMultiBoxLoss (SSD multibox distillation loss).

Data-parallel over the batch dim across 8 NeuronCores; host sums the 8x16
partial columns and performs the final division by N.

v3 design:
  - host casts conf/loc inputs to bf16: HBM traffic 49 MB -> 26 MB per core
  - PE does the conf[p, ct_p] one-hot trace matmul in bf16
  - sumexp per prior: fold level 1 on DVE (bf16 2x), levels 2-6 on the
    otherwise-idle Pool engine, f32 finals on DVE
  - all Ln batched at the end; smooth-L1 in the exact
    0.5*min(u,1)^2 + (u - min(u,1)) form (masked elements contribute 0)
  - hard-negative top-k: lcm is re-partitioned via an SBUF->SBUF DMA into a
    (row, replica) chunk layout so each partition's threshold is a
    per-partition tensor_scalar operand (4x DVE mode, fused count accum);
    the exact pass is a single max-accum per tensor:
    topk = sum(max(v, lo)) + (k - 8832) * lo
  - prior tiling: partition p holds priors [69p, 69p+69); partition 126 is
    end-aligned (priors 8663..8731, first 31 slots masked as duplicates of
    p125's tail); partition 127 is zero-filled by a small const DMA
"""

import os
import sys

sys.path.insert(0, "/opt/trn_rl_repo")

from contextlib import ExitStack

import ml_dtypes
import numpy as np

import concourse.bass as bass
import concourse.bacc as bacc
import concourse.mybir as mybir
import concourse.tile as tile
from concourse.bass_utils import run_bass_kernel_spmd

F32 = mybir.dt.float32
BF16 = mybir.dt.bfloat16
I32 = mybir.dt.int32
ALU = mybir.AluOpType
ACT = mybir.ActivationFunctionType

# ---- problem geometry (hardcoded) ----
B, P, C = 64, 8732, 81
NCORES = 8
R = B // NCORES            # 8 batch rows per core
NT = 69                    # priors per partition per row (126*69+38 = 8732)
P_FULL = 126 * NT          # 8694 priors on partitions 0..125
TAIL = P - P_FULL          # 38 real priors on partition 126
TAIL_OFF = NT - TAIL       # 31 duplicate slots at the start of p126
RC = R * NT                # 552 row-tiled columns
NREP = 16                  # search chunk replicas per row (16*552 = 8832 slots)
NSLOT = 128 * NT // NREP   # 552 lcm slots per chunk partition
LTT = 546                  # flat loc tiling: 128 * 546 >= R * P
LF = LTT * 4               # 2184
PADN = 128 * LTT - R * P
NPART = 16
NITER = 7                  # binary search iterations (thresholds stay
HI_INIT = 16.0             # bf16-exact: dyadics with <= 8 mantissa bits)

# partials columns
(COL_BT, COL_BS, COL_AT, COL_AS, COL_CT, COL_CS, COL_DT, COL_DS,
 COL_LT, COL_LS, COL_TKT, COL_TKS, COL_NP) = range(13)

STAGE = int(os.environ.get("K_STAGE", "9"))
# fold levels handled by the Pool engine (per-level start column widths)
POOL_LVLS = set(
    int(x) for x in os.environ.get("K_POOL_LVLS", "").split(",") if x)
# tensor_scalar fused accumulate (TensorScalarPtrReduce) for search sums
TS_ACCUM = os.environ.get("K_TS_ACCUM", "1") == "1"


def build_nc():
    nc = bacc.Bacc("TRN2", target_bir_lowering=False, debug=False,
                   num_devices=NCORES)

    conf_T = nc.declare_dram_parameter("conf_T", [R, P, C], BF16, isOutput=False)
    conf_S = nc.declare_dram_parameter("conf_S", [R, P, C], BF16, isOutput=False)
    loc_T = nc.declare_dram_parameter("loc_T", [128 * LTT, 4], BF16, isOutput=False)
    loc_S = nc.declare_dram_parameter("loc_S", [128 * LTT, 4], BF16, isOutput=False)
    loc_t = nc.declare_dram_parameter("loc_t", [128 * LTT, 4], BF16, isOutput=False)
    ctbf_p = nc.declare_dram_parameter("ctbf", [128, RC], BF16, isOutput=False)
    ctfl_p = nc.declare_dram_parameter("ctfl", [128, LTT], BF16, isOutput=False)
    eye81_p = nc.declare_dram_parameter("eye81", [81, 81], BF16, isOutput=False)
    g8_p = nc.declare_dram_parameter("g8", [128, 8], F32, isOutput=False)
    g8t_p = nc.declare_dram_parameter("g8t", [8, 128], F32, isOutput=False)
    ones_p = nc.declare_dram_parameter("ones128", [128, 1], F32, isOutput=False)
    zpad_p = nc.declare_dram_parameter("zpad", [1, NT * C], BF16, isOutput=False)
    scr_p = nc.declare_dram_parameter("scr", [2, 128 * NSLOT], BF16, isOutput=False)
    out_p = nc.declare_dram_parameter("out", [1, NPART], F32, isOutput=True)

    with tile.TileContext(nc) as tc, ExitStack() as ctx:
        cpool = ctx.enter_context(tc.tile_pool(name="consts", bufs=1))
        pers = ctx.enter_context(tc.tile_pool(name="pers", bufs=1))
        small = ctx.enter_context(tc.tile_pool(name="small", bufs=1))
        pool_c = ctx.enter_context(tc.tile_pool(name="conf", bufs=2))
        pool_e = ctx.enter_context(tc.tile_pool(name="expx", bufs=2))
        pool_q = ctx.enter_context(tc.tile_pool(name="eq", bufs=2))
        pool_f = ctx.enter_context(tc.tile_pool(name="fold", bufs=2))
        psum = ctx.enter_context(tc.tile_pool(name="ps", bufs=4, space="PSUM"))
        pstr = ctx.enter_context(tc.tile_pool(name="tr", bufs=1, space="PSUM"))

        # ---- persistent tensors ----
        ctbf = pers.tile([128, RC], BF16)
        posf = pers.tile([128, RC], BF16)
        valid = pers.tile([128, RC], BF16)
        ominus = pers.tile([128, RC], BF16)
        sumexp2 = pers.tile([128, 2, RC], F32)      # becomes lse2 in place
        conf02 = pers.tile([128, 2, RC], F32)
        lcm2 = {xi: pers.tile([128, RC], BF16, name=f"lcm{xi}")
                for xi in range(2)}
        lcmc = {xi: pers.tile([128, NSLOT], BF16, name=f"lcmc{xi}")
                for xi in range(2)}                 # chunked search layout
        sjc = {xi: pers.tile([128, NSLOT], BF16, name=f"sjc{xi}")
               for xi in range(2)}
        sjunk2 = pers.tile([128, 2, RC], F32)
        partials = pers.tile([128, NPART], F32)

        ctflb = pers.tile([128, LTT], BF16)
        posml = pers.tile([128, LTT], BF16)
        posml4 = pers.tile([128, LF], BF16)
        locsb = {n: pers.tile([128, LF], BF16, name=f"loc{n}")
                 for n in ("T", "S", "t")}
        lwu = pers.tile([128, LF], BF16)
        lws = pers.tile([128, LF], BF16)
        lwd = pers.tile([128, LF], BF16)

        # ---- constants ----
        eye81 = cpool.tile([81, 81], BF16)
        g8 = cpool.tile([128, 8], F32)
        g8t = cpool.tile([8, 128], F32)
        ones128 = cpool.tile([128, 1], F32)
        iota81 = cpool.tile([128, 81], BF16)

        def emit_consts():
            nc.sync.dma_start(out=eye81[:, :], in_=eye81_p.ap())
            nc.sync.dma_start(out=g8[:, :], in_=g8_p.ap())
            nc.sync.dma_start(out=g8t[:, :], in_=g8t_p.ap())
            nc.sync.dma_start(out=ones128[:, :], in_=ones_p.ap())
            nc.gpsimd.iota(iota81[:, :], pattern=[[1, 81]], base=0,
                           channel_multiplier=0,
                           allow_small_or_imprecise_dtypes=True)
            nc.gpsimd.memset(partials[:, :], 0.0)
            nc.sync.dma_start(out=ctbf[:, :], in_=ctbf_p.ap())
            nc.vector.tensor_scalar(out=posf[:, :], in0=ctbf[:, :],
                                    scalar1=0.5, scalar2=None, op0=ALU.is_gt)
            nc.vector.tensor_scalar(out=valid[:, :], in0=ctbf[:, :],
                                    scalar1=-0.5, scalar2=None, op0=ALU.is_gt)
            nc.vector.tensor_tensor(out=ominus[:, :], in0=valid[:, :],
                                    in1=posf[:, :], op=ALU.subtract)

        def emit_loc_dmas():
            nc.sync.dma_start(out=ctflb[:, :], in_=ctfl_p.ap())
            for name, param in (("T", loc_T), ("S", loc_S), ("t", loc_t)):
                nc.sync.dma_start(
                    out=locsb[name][:, :],
                    in_=param.ap().rearrange("(p j) f -> p (j f)", j=LTT))

        def emit_loc_chain():
            # Pool takes the bulk elementwise ops (it is otherwise idle);
            # DVE keeps only min and the final reduce, ACT does abs/square.
            nc.vector.tensor_scalar(out=posml[:, :], in0=ctflb[:, :],
                                    scalar1=0.5, scalar2=None, op0=ALU.is_gt)
            # posml4[p, 4j+f] = posml[p, j]
            nc.gpsimd.tensor_copy(
                out=posml4[:, :].rearrange("p (j f) -> p j f", f=4),
                in_=posml[:, :].unsqueeze(2).broadcast_to((128, LTT, 4)))
            for x, col in (("T", COL_LT), ("S", COL_LS)):
                nc.gpsimd.tensor_tensor(out=lwd[:, :], in0=locsb[x][:, :],
                                        in1=locsb["t"][:, :], op=ALU.subtract)
                nc.gpsimd.tensor_tensor(out=lwu[:, :], in0=lwd[:, :],
                                        in1=posml4[:, :], op=ALU.mult)
                nc.scalar.activation(out=lwu[:, :], in_=lwu[:, :], func=ACT.Abs)
                nc.vector.tensor_scalar(out=lws[:, :], in0=lwu[:, :],
                                        scalar1=1.0, scalar2=None, op0=ALU.min)
                # lwd = 0.5 * s^2
                nc.scalar.activation(out=lwd[:, :], in_=lws[:, :],
                                     func=ACT.Square,
                                     scale=float(1.0 / np.sqrt(2.0)))
                nc.vector.tensor_tensor(out=lwu[:, :], in0=lwu[:, :],
                                        in1=lws[:, :], op=ALU.subtract)
                nc.vector.tensor_tensor(out=lwd[:, :], in0=lwd[:, :],
                                        in1=lwu[:, :], op=ALU.add)
                nc.vector.tensor_reduce(out=partials[:, col:col + 1],
                                        in_=lwd[:, :],
                                        axis=mybir.AxisListType.X, op=ALU.add)

        # ---- conf streaming loop ----
        pstr2 = pstr.tile([81, 2, C], F32)
        nmm = 0
        total_mm = R * NT

        for r in range(R):
            rc = r * NT
            ctile = pool_c.tile([128, 2, NT, C], BF16, name="ctile")
            for xi, (x, param) in enumerate((("T", conf_T), ("S", conf_S))):
                rowv = param.ap()[r, :, :]
                nc.sync.dma_start(
                    out=ctile[0:126, xi, :, :],
                    in_=rowv[0:P_FULL, :].rearrange("(p t) c -> p t c", t=NT))
                nc.sync.dma_start(
                    out=ctile[126:127, xi, :, :],
                    in_=rowv[P - NT:P, :].unsqueeze(0))
                if r < 2:
                    # p127 is never written by the row DMAs: fill with zeros
                    # once per buffer slot so exp() stays finite
                    nc.sync.dma_start(
                        out=ctile[127:128, xi, :, :],
                        in_=zpad_p.ap().rearrange("o (t c) -> o t c", c=C))
            if r == 0:
                emit_consts()
            if r == 1:
                emit_loc_dmas()

            ex = pool_e.tile([128, 2, NT, C], BF16, name="ex")
            for xi in range(2):
                nc.scalar.activation(out=ex[:, xi, :, :],
                                     in_=ctile[:, xi, :, :], func=ACT.Exp)

            # one-hot eq (broadcast operands: no DVE fast path exists)
            eq = pool_q.tile([128, NT, C], BF16, name="eq")
            if STAGE >= 3:
                nc.vector.tensor_tensor(
                    out=eq[:, :, :],
                    in0=iota81[:, :].unsqueeze(1).broadcast_to((128, NT, C)),
                    in1=ctbf[:, rc:rc + NT].unsqueeze(2).broadcast_to(
                        (128, NT, C)),
                    op=ALU.is_equal)

                # B trace: psum[m, xi, c] += sum_p eq[p,t,m] * conf[p,xi,t,c]
                for t in range(NT):
                    nc.tensor.matmul(
                        pstr2[:, :, :],
                        lhsT=eq[:, t, :],
                        rhs=ctile[:, :, t, :],
                        start=(nmm == 0), stop=(nmm == total_mm - 1))
                    nmm += 1

            # sumexp fold-tree: level 1 on DVE (bf16 2x), rest on Pool
            if STAGE >= 4:
                fold = pool_f.tile([128, 2, NT, 40], BF16, name="fold")
                for xi in range(2):
                    f = fold[:, xi, :, :]
                    e = ex[:, xi, :, :]
                    lvls = [(1, 40, e), (2, 20, None), (3, 10, None),
                            (4, 5, None), (5, 2, None), (6, 1, None)]
                    for lvl, w, src in lvls:
                        eng = nc.gpsimd if lvl in POOL_LVLS else nc.vector
                        if src is not None:
                            eng.tensor_tensor(out=f[:, :, 0:w],
                                              in0=src[:, :, 0:40],
                                              in1=src[:, :, 40:80], op=ALU.add)
                        else:
                            eng.tensor_tensor(out=f[:, :, 0:w],
                                              in0=f[:, :, 0:w],
                                              in1=f[:, :, w:2 * w], op=ALU.add)
                    se = sumexp2[:, xi, rc:rc + NT]
                    nc.vector.tensor_tensor(out=se, in0=f[:, :, 0],
                                            in1=f[:, :, 4], op=ALU.add)
                    nc.vector.tensor_tensor(out=se, in0=se,
                                            in1=e[:, :, 80], op=ALU.add)
                    nc.gpsimd.tensor_copy(out=conf02[:, xi, rc:rc + NT],
                                          in_=ctile[:, xi, :, 0])

            if r == 3:
                emit_loc_chain()

        # num_pos per row -> k (only needed by the tail search)
        npp = small.tile([128, 8], F32)
        nc.vector.tensor_reduce(out=npp[:, :],
                                in_=posf[:, :].rearrange("p (r t) -> p r t", r=R),
                                axis=mybir.AxisListType.X, op=ALU.add)
        ps_np = psum.tile([8, 1], F32, tag="ps")
        nc.tensor.matmul(ps_np[:, :], lhsT=npp[:, :], rhs=ones128[:, :],
                         start=True, stop=True)
        np8 = small.tile([8, 1], F32)
        nc.vector.tensor_copy(out=np8[:, :], in_=ps_np[:, :])
        k82 = small.tile([8, 2], F32)
        nc.vector.tensor_scalar(out=k82[:, 0:1], in0=np8[:, :], scalar1=3.0,
                                scalar2=float(P - 1), op0=ALU.mult, op1=ALU.min)
        nc.vector.tensor_copy(out=k82[:, 1:2], in_=k82[:, 0:1])
        nc.vector.tensor_copy(out=partials[0:8, COL_NP:COL_NP + 1],
                              in_=np8[:, :])

        # ---- tail: lse, lcm, epilogue sums ----
        if STAGE >= 5:
            # lse in place of sumexp (one Ln, one table load)
            nc.scalar.activation(out=sumexp2[:, :, :], in_=sumexp2[:, :, :],
                                 func=ACT.Ln)
            lse2 = sumexp2
            for xi in range(2):
                nc.vector.tensor_tensor(out=lcm2[xi][:, :],
                                        in0=lse2[:, xi, :],
                                        in1=conf02[:, xi, :], op=ALU.subtract)
                nc.vector.tensor_tensor(out=lcm2[xi][:, :],
                                        in0=lcm2[xi][:, :], in1=ominus[:, :],
                                        op=ALU.mult)
                # re-partition lcm into the chunk layout via a DRAM bounce:
                # chunk partition q = 16r + k holds source partitions
                # 8k..8k+7 of row r.  (A direct SBUF->SBUF partition-split
                # rearrange lowers to out-of-bounds free-dim strides.)
                nc.sync.dma_start(
                    out=scr_p.ap()[xi, :].rearrange(
                        "(r k i t) -> k i r t", r=R, k=NREP, i=8),
                    in_=lcm2[xi][:, :].rearrange(
                        "(k i) (r t) -> (k i) r t", k=NREP, r=R))
                nc.sync.dma_start(
                    out=lcmc[xi][:, :],
                    in_=scr_p.ap()[xi, :].rearrange("(q j) -> q j", q=128))

            # A = sum(lse*posf), C = sum(conf0*valid), D = sum(conf0*posf)
            # Pool mults into three distinct scratch slabs (lwd/lwu are dead
            # after the loc chain) so the DVE reduces don't serialize them.
            sj3 = (sjunk2[:, :, :],
                   lwd[:, 0:2 * RC].rearrange("p (x c) -> p x c", x=2),
                   lwu[:, 0:2 * RC].rearrange("p (x c) -> p x c", x=2))
            for src, mask, (c0, c1), sj in (
                    (lse2, posf, (COL_AT, COL_AS), sj3[0]),
                    (conf02, valid, (COL_CT, COL_CS), sj3[1]),
                    (conf02, posf, (COL_DT, COL_DS), sj3[2])):
                nc.gpsimd.tensor_tensor(
                    out=sj, in0=src[:, :, :],
                    in1=mask[:, :].unsqueeze(1).broadcast_to((128, 2, RC)),
                    op=ALU.mult)
                assert c1 == c0 + 1
                nc.vector.tensor_reduce(out=partials[:, c0:c1 + 1],
                                        in_=sj,
                                        axis=mybir.AxisListType.X, op=ALU.add)

            # B extraction from the PSUM trace
            nc.vector.tensor_tensor(
                out=sjunk2[0:81, :, 0:81], in0=pstr2[:, :, :],
                in1=eye81[:, :].unsqueeze(1).broadcast_to((81, 2, 81)),
                op=ALU.mult)
            nc.vector.tensor_reduce(out=partials[0:81, COL_BT:COL_BS + 1],
                                    in_=sjunk2[0:81, :, 0:81],
                                    axis=mybir.AxisListType.X, op=ALU.add)

        # ---- chunked binary search for per-row top-k thresholds ----
        if STAGE >= 6:
            lo82 = small.tile([8, 2], F32)
            hi82 = small.tile([8, 2], F32)
            tm82 = small.tile([8, 2], F32)
            s82 = small.tile([8, 2], F32)
            ge82 = small.tile([8, 2], I32)
            gei82 = small.tile([8, 2], I32)
            trep2 = small.tile([128, 2], F32)
            cnt2 = small.tile([128, 2], F32)
            sm2 = small.tile([128, 2], F32)
            tk82 = small.tile([8, 2], F32)

            def bcast82(vec82):
                psT = psum.tile([128, 2], F32, name="psT", tag="ps")
                nc.tensor.matmul(psT[:, :], lhsT=g8t[:, :], rhs=vec82[:, :],
                                 start=True, stop=True)
                nc.vector.tensor_copy(out=trep2[:, :], in_=psT[:, :])

            nc.gpsimd.memset(lo82[:, :], 0.0)
            nc.gpsimd.memset(hi82[:, :], HI_INIT)
            for it in range(NITER):
                nc.vector.tensor_tensor(out=tm82[:, :], in0=lo82[:, :],
                                        in1=hi82[:, :], op=ALU.add)
                nc.vector.tensor_scalar(out=tm82[:, :], in0=tm82[:, :],
                                        scalar1=0.5, scalar2=None, op0=ALU.mult)
                bcast82(tm82)
                for xi in range(2):
                    if TS_ACCUM:
                        nc.vector.tensor_scalar(
                            out=sjc[xi][:, :], in0=lcmc[xi][:, :],
                            scalar1=trep2[:, xi:xi + 1], scalar2=0.0,
                            op0=ALU.is_gt, op1=ALU.add,
                            accum_out=cnt2[:, xi:xi + 1])
                    else:
                        nc.vector.tensor_scalar(
                            out=sjc[xi][:, :], in0=lcmc[xi][:, :],
                            scalar1=trep2[:, xi:xi + 1], scalar2=None,
                            op0=ALU.is_gt)
                        nc.vector.tensor_reduce(
                            out=cnt2[:, xi:xi + 1], in_=sjc[xi][:, :],
                            axis=mybir.AxisListType.X, op=ALU.add)
                psN = psum.tile([8, 2], F32, name="psN", tag="ps")
                nc.tensor.matmul(psN[:, :], lhsT=g8[:, :], rhs=cnt2[:, :],
                                 start=True, stop=True)
                nc.vector.tensor_copy(out=s82[:, :], in_=psN[:, :])
                nc.vector.tensor_tensor(out=ge82[:, :], in0=s82[:, :],
                                        in1=k82[:, :], op=ALU.is_ge)
                nc.vector.copy_predicated(out=lo82[:, :], mask=ge82[:, :],
                                          data=tm82[:, :])
                nc.vector.tensor_scalar(out=gei82[:, :], in0=ge82[:, :],
                                        scalar1=1, scalar2=None,
                                        op0=ALU.bitwise_xor)
                nc.vector.copy_predicated(out=hi82[:, :], mask=gei82[:, :],
                                          data=tm82[:, :])

            # exact pass: topk = sum(max(v, lo)) + (k - 8832) * lo
            bcast82(lo82)
            for xi in range(2):
                if TS_ACCUM:
                    nc.vector.tensor_scalar(
                        out=sjc[xi][:, :], in0=lcmc[xi][:, :],
                        scalar1=trep2[:, xi:xi + 1], scalar2=0.0,
                        op0=ALU.max, op1=ALU.add,
                        accum_out=sm2[:, xi:xi + 1])
                else:
                    nc.vector.tensor_scalar(
                        out=sjc[xi][:, :], in0=lcmc[xi][:, :],
                        scalar1=trep2[:, xi:xi + 1], scalar2=None,
                        op0=ALU.max)
                    nc.vector.tensor_reduce(
                        out=sm2[:, xi:xi + 1], in_=sjc[xi][:, :],
                        axis=mybir.AxisListType.X, op=ALU.add)
            psE = psum.tile([8, 2], F32, name="psE", tag="ps")
            nc.tensor.matmul(psE[:, :], lhsT=g8[:, :], rhs=sm2[:, :],
                             start=True, stop=True)
            nc.vector.tensor_copy(out=tk82[:, :], in_=psE[:, :])
            nc.vector.tensor_scalar(out=s82[:, :], in0=k82[:, :],
                                    scalar1=float(NREP * NSLOT), scalar2=None,
                                    op0=ALU.subtract)
            nc.vector.tensor_tensor(out=s82[:, :], in0=s82[:, :],
                                    in1=lo82[:, :], op=ALU.mult)
            nc.vector.tensor_tensor(out=tk82[:, :], in0=tk82[:, :],
                                    in1=s82[:, :], op=ALU.add)
            nc.vector.tensor_copy(out=partials[0:8, COL_TKT:COL_TKT + 1],
                                  in_=tk82[:, 0:1])
            nc.vector.tensor_copy(out=partials[0:8, COL_TKS:COL_TKS + 1],
                                  in_=tk82[:, 1:2])

        # ---- final partition reduce of partials -> out ----
        psF = psum.tile([1, NPART], F32, name="psF", tag="ps")
        nc.tensor.matmul(psF[:, :], lhsT=ones128[:, :], rhs=partials[:, :],
                         start=True, stop=True)
        fin = small.tile([1, NPART], F32)
        nc.vector.tensor_copy(out=fin[:, :], in_=psF[:, :])
        nc.sync.dma_start(out=out_p.ap(), in_=fin[:, :])
    nc.finalize()
    return nc


_NC_CACHE = None


def _get_nc():
    global _NC_CACHE
    if _NC_CACHE is None:
        _NC_CACHE = build_nc()
    return _NC_CACHE


def _host_consts():
    eye81 = np.eye(81, dtype=ml_dtypes.bfloat16)
    g8 = np.zeros((128, 8), np.float32)
    for p in range(128):
        g8[p, p // NREP] = 1.0
    g8t = np.ascontiguousarray(g8.T)
    ones128 = np.ones((128, 1), np.float32)
    zpad = np.zeros((1, NT * C), ml_dtypes.bfloat16)
    return eye81, g8, g8t, ones128, zpad


def _ct_row_tiled(ct_rows: np.ndarray) -> np.ndarray:
    """[R, P] int -> [128, R*NT] bf16 row-tiled, pads/duplicates = -1."""
    out = np.full((128, RC), -1.0, np.float32)
    for r in range(R):
        out[0:126, r * NT:(r + 1) * NT] = ct_rows[r, 0:P_FULL].reshape(126, NT)
        out[126, r * NT + TAIL_OFF:(r + 1) * NT] = ct_rows[r, P_FULL:P]
    return out.astype(ml_dtypes.bfloat16)


def _ct_flat(ct_rows: np.ndarray) -> np.ndarray:
    flat = np.full(128 * LTT, -1.0, np.float32)
    flat[:R * P] = ct_rows.reshape(-1)
    return flat.reshape(128, LTT).astype(ml_dtypes.bfloat16)


def _build_in_maps(inputs):
    conf_T = np.asarray(inputs["conf_dataT"], np.float32)
    conf_S = np.asarray(inputs["conf_dataS"], np.float32)
    loc_T = np.asarray(inputs["loc_dataT"], np.float32)
    loc_S = np.asarray(inputs["loc_dataS"], np.float32)
    loc_t = np.asarray(inputs["loc_t"], np.float32)
    ct = np.asarray(inputs["conf_t"], np.int32)

    def _padloc(a):
        flat = a.reshape(R * P, 4)
        return np.ascontiguousarray(
            np.pad(flat, ((0, PADN), (0, 0)))).astype(ml_dtypes.bfloat16)

    eye81, g8, g8t, ones128, zpad = _host_consts()
    in_maps = []
    for d in range(NCORES):
        sl = slice(d * R, (d + 1) * R)
        ctsl = ct[sl]
        in_maps.append({
            "conf_T": np.ascontiguousarray(conf_T[sl]).astype(ml_dtypes.bfloat16),
            "conf_S": np.ascontiguousarray(conf_S[sl]).astype(ml_dtypes.bfloat16),
            "loc_T": _padloc(loc_T[sl]), "loc_S": _padloc(loc_S[sl]),
            "loc_t": _padloc(loc_t[sl]),
            "ctbf": _ct_row_tiled(ctsl),
            "ctfl": _ct_flat(ctsl),
            "eye81": eye81, "g8": g8, "g8t": g8t,
            "ones128": ones128, "zpad": zpad,
            "scr": np.zeros((2, 128 * NSLOT), ml_dtypes.bfloat16),
        })
    return in_maps


def _combine(parts):
    S = parts.astype(np.float64).sum(axis=0)
    loss_cT = S[COL_AT] - S[COL_BT] + S[COL_CT] - S[COL_DT] + S[COL_TKT]
    loss_cS = S[COL_AS] - S[COL_BS] + S[COL_CS] - S[COL_DS] + S[COL_TKS]
    N = S[COL_NP]
    return np.array([S[COL_LT] / N, loss_cT / N, S[COL_LS] / N, loss_cS / N],
                    np.float32)


def run_on_hw(inputs, trace=False, **kw):
    nc = _get_nc()
    in_maps = _build_in_maps(inputs)
    res = run_bass_kernel_spmd(nc, in_maps, core_ids=list(range(NCORES)),
                               trace=trace, **kw)
    parts = np.stack([np.asarray(r["out"]).reshape(NPART) for r in res.results])
    return _combine(parts), res


def kernel(**inputs) -> np.ndarray:
    out, _ = run_on_hw(inputs, trace=False)
    return out


# revision 43
# speedup vs baseline: 1.4365x; 1.0571x over previous
"""Trainium2 Bass kernel for nn_AdaptiveMultiBoxLoss (SSD multibox distillation loss).

Data-parallel over the batch dim across 8 NeuronCores; host sums the 8x16
partial columns and performs the final division by N.

v3 design:
  - host casts conf/loc inputs to bf16: HBM traffic 49 MB -> 26 MB per core
  - PE does the conf[p, ct_p] one-hot trace matmul in bf16
  - sumexp per prior: fold level 1 on DVE (bf16 2x), levels 2-6 on the
    otherwise-idle Pool engine, f32 finals on DVE
  - all Ln batched at the end; smooth-L1 in the exact
    0.5*min(u,1)^2 + (u - min(u,1)) form (masked elements contribute 0)
  - hard-negative top-k: lcm is re-partitioned via an SBUF->SBUF DMA into a
    (row, replica) chunk layout so each partition's threshold is a
    per-partition tensor_scalar operand (4x DVE mode, fused count accum);
    the exact pass is a single max-accum per tensor:
    topk = sum(max(v, lo)) + (k - 8832) * lo
  - prior tiling: partition p holds priors [69p, 69p+69); partition 126 is
    end-aligned (priors 8663..8731, first 31 slots masked as duplicates of
    p125's tail); partition 127 is zero-filled by a small const DMA
"""

import os
import sys

sys.path.insert(0, "/opt/trn_rl_repo")

from contextlib import ExitStack

import ml_dtypes
import numpy as np

import concourse.bass as bass
import concourse.bacc as bacc
import concourse.mybir as mybir
import concourse.tile as tile
from concourse.bass_utils import run_bass_kernel_spmd

F32 = mybir.dt.float32
BF16 = mybir.dt.bfloat16
I32 = mybir.dt.int32
ALU = mybir.AluOpType
ACT = mybir.ActivationFunctionType

# ---- problem geometry (hardcoded) ----
B, P, C = 64, 8732, 81
NCORES = 8
R = B // NCORES            # 8 batch rows per core
NT = 69                    # priors per partition per row (126*69+38 = 8732)
P_FULL = 126 * NT          # 8694 priors on partitions 0..125
TAIL = P - P_FULL          # 38 real priors on partition 126
TAIL_OFF = NT - TAIL       # 31 duplicate slots at the start of p126
RC = R * NT                # 552 row-tiled columns
NREP = 16                  # search chunk replicas per row (16*552 = 8832 slots)
NSLOT = 128 * NT // NREP   # 552 lcm slots per chunk partition
LTT = 546                  # flat loc tiling: 128 * 546 >= R * P
LF = LTT * 4               # 2184
PADN = 128 * LTT - R * P
NPART = 16
NITER = 6                  # binary search iterations (thresholds stay
HI_INIT = 16.0             # bf16-exact: dyadics with <= 8 mantissa bits)

# partials columns
(COL_BT, COL_BS, COL_AT, COL_AS, COL_CT, COL_CS, COL_DT, COL_DS,
 COL_LT, COL_LS, COL_TKT, COL_TKS, COL_NP) = range(13)

STAGE = int(os.environ.get("K_STAGE", "9"))
# fold levels handled by the Pool engine (per-level start column widths)
POOL_LVLS = set(
    int(x) for x in os.environ.get("K_POOL_LVLS", "").split(",") if x)
# tensor_scalar fused accumulate (TensorScalarPtrReduce) for search sums
TS_ACCUM = os.environ.get("K_TS_ACCUM", "1") == "1"


def build_nc():
    nc = bacc.Bacc("TRN2", target_bir_lowering=False, debug=False,
                   num_devices=NCORES)

    conf_T = nc.declare_dram_parameter("conf_T", [R, P, C], BF16, isOutput=False)
    conf_S = nc.declare_dram_parameter("conf_S", [R, P, C], BF16, isOutput=False)
    loc_T = nc.declare_dram_parameter("loc_T", [128 * LTT, 4], BF16, isOutput=False)
    loc_S = nc.declare_dram_parameter("loc_S", [128 * LTT, 4], BF16, isOutput=False)
    loc_t = nc.declare_dram_parameter("loc_t", [128 * LTT, 4], BF16, isOutput=False)
    ctbf_p = nc.declare_dram_parameter("ctbf", [128, RC], BF16, isOutput=False)
    ctfl_p = nc.declare_dram_parameter("ctfl", [128, LTT], BF16, isOutput=False)
    eye81_p = nc.declare_dram_parameter("eye81", [81, 81], BF16, isOutput=False)
    g8_p = nc.declare_dram_parameter("g8", [128, 8], F32, isOutput=False)
    g8t_p = nc.declare_dram_parameter("g8t", [8, 128], F32, isOutput=False)
    ones_p = nc.declare_dram_parameter("ones128", [128, 1], F32, isOutput=False)
    zpad_p = nc.declare_dram_parameter("zpad", [1, NT * C], BF16, isOutput=False)
    scr_p = nc.declare_dram_parameter("scr", [2, 128 * NSLOT], BF16, isOutput=False)
    out_p = nc.declare_dram_parameter("out", [1, NPART], F32, isOutput=True)

    with tile.TileContext(nc) as tc, ExitStack() as ctx:
        cpool = ctx.enter_context(tc.tile_pool(name="consts", bufs=1))
        pers = ctx.enter_context(tc.tile_pool(name="pers", bufs=1))
        small = ctx.enter_context(tc.tile_pool(name="small", bufs=1))
        pool_c = ctx.enter_context(tc.tile_pool(name="conf", bufs=3))
        pool_e = ctx.enter_context(tc.tile_pool(name="expx", bufs=2))
        pool_q = ctx.enter_context(tc.tile_pool(name="eq", bufs=2))
        psum = ctx.enter_context(tc.tile_pool(name="ps", bufs=4, space="PSUM"))
        pstr = ctx.enter_context(tc.tile_pool(name="tr", bufs=1, space="PSUM"))

        # ---- persistent tensors ----
        ctbf = pers.tile([128, RC], BF16)
        posf = pers.tile([128, RC], BF16)
        valid = pers.tile([128, RC], BF16)
        ominus = pers.tile([128, RC], BF16)
        sumexp2 = pers.tile([128, 2, RC], F32)      # becomes lse2 in place
        conf02 = pers.tile([128, 2, RC], F32)
        lcm2 = {xi: pers.tile([128, RC], BF16, name=f"lcm{xi}")
                for xi in range(2)}
        lcmc = {xi: pers.tile([128, NSLOT], BF16, name=f"lcmc{xi}")
                for xi in range(2)}                 # chunked search layout
        sjc = {xi: pers.tile([128, NSLOT], BF16, name=f"sjc{xi}")
               for xi in range(2)}
        sjunk2 = pers.tile([128, 2, RC], F32)
        partials = pers.tile([128, NPART], F32)

        ctflb = pers.tile([128, LTT], BF16)
        posml = pers.tile([128, LTT], BF16)
        posml4 = pers.tile([128, LF], BF16)
        locsb = {n: pers.tile([128, LF], BF16, name=f"loc{n}")
                 for n in ("T", "S", "t")}
        lwu = pers.tile([128, LF], BF16)
        lws = pers.tile([128, LF], BF16)
        lwd = pers.tile([128, LF], BF16)

        # ---- constants ----
        eye81 = cpool.tile([81, 81], BF16)
        g8 = cpool.tile([128, 8], F32)
        g8t = cpool.tile([8, 128], F32)
        ones128 = cpool.tile([128, 1], F32)
        iota81 = cpool.tile([128, 81], BF16)

        def emit_consts():
            nc.sync.dma_start(out=eye81[:, :], in_=eye81_p.ap())
            nc.sync.dma_start(out=g8[:, :], in_=g8_p.ap())
            nc.sync.dma_start(out=g8t[:, :], in_=g8t_p.ap())
            nc.sync.dma_start(out=ones128[:, :], in_=ones_p.ap())
            nc.gpsimd.iota(iota81[:, :], pattern=[[1, 81]], base=0,
                           channel_multiplier=0,
                           allow_small_or_imprecise_dtypes=True)
            nc.gpsimd.memset(partials[:, :], 0.0)
            nc.sync.dma_start(out=ctbf[:, :], in_=ctbf_p.ap())
            nc.vector.tensor_scalar(out=posf[:, :], in0=ctbf[:, :],
                                    scalar1=0.5, scalar2=None, op0=ALU.is_gt)
            nc.vector.tensor_scalar(out=valid[:, :], in0=ctbf[:, :],
                                    scalar1=-0.5, scalar2=None, op0=ALU.is_gt)
            nc.vector.tensor_tensor(out=ominus[:, :], in0=valid[:, :],
                                    in1=posf[:, :], op=ALU.subtract)

        def emit_loc_dmas():
            nc.sync.dma_start(out=ctflb[:, :], in_=ctfl_p.ap())
            for name, param in (("T", loc_T), ("S", loc_S), ("t", loc_t)):
                nc.sync.dma_start(
                    out=locsb[name][:, :],
                    in_=param.ap().rearrange("(p j) f -> p (j f)", j=LTT))

        def emit_loc_chain():
            # Pool takes the bulk elementwise ops (it is otherwise idle);
            # DVE keeps only min and the final reduce, ACT does abs/square.
            nc.vector.tensor_scalar(out=posml[:, :], in0=ctflb[:, :],
                                    scalar1=0.5, scalar2=None, op0=ALU.is_gt)
            # posml4[p, 4j+f] = posml[p, j] (ACT copy: cheap, off the DVE)
            nc.scalar.activation(
                out=posml4[:, :].rearrange("p (j f) -> p j f", f=4),
                in_=posml[:, :].unsqueeze(2).broadcast_to((128, LTT, 4)),
                func=ACT.Copy)
            for x, col in (("T", COL_LT), ("S", COL_LS)):
                nc.gpsimd.tensor_tensor(out=lwd[:, :], in0=locsb[x][:, :],
                                        in1=locsb["t"][:, :], op=ALU.subtract)
                nc.gpsimd.tensor_tensor(out=lwu[:, :], in0=lwd[:, :],
                                        in1=posml4[:, :], op=ALU.mult)
                nc.scalar.activation(out=lwu[:, :], in_=lwu[:, :], func=ACT.Abs)
                nc.vector.tensor_scalar(out=lws[:, :], in0=lwu[:, :],
                                        scalar1=1.0, scalar2=None, op0=ALU.min)
                # lwd = 0.5 * s^2
                nc.scalar.activation(out=lwd[:, :], in_=lws[:, :],
                                     func=ACT.Square,
                                     scale=float(1.0 / np.sqrt(2.0)))
                nc.vector.tensor_tensor(out=lwu[:, :], in0=lwu[:, :],
                                        in1=lws[:, :], op=ALU.subtract)
                nc.vector.tensor_tensor(out=lwd[:, :], in0=lwd[:, :],
                                        in1=lwu[:, :], op=ALU.add)
                nc.vector.tensor_reduce(out=partials[:, col:col + 1],
                                        in_=lwd[:, :],
                                        axis=mybir.AxisListType.X, op=ALU.add)

        # ---- conf streaming loop ----
        pstr2 = pstr.tile([81, 2, C], F32)
        nmm = 0
        total_mm = R * NT

        for r in range(R):
            rc = r * NT
            ctile = pool_c.tile([128, 2, NT, C], BF16, name="ctile")
            for xi, (x, param) in enumerate((("T", conf_T), ("S", conf_S))):
                rowv = param.ap()[r, :, :]
                nc.sync.dma_start(
                    out=ctile[0:126, xi, :, :],
                    in_=rowv[0:P_FULL, :].rearrange("(p t) c -> p t c", t=NT))
                nc.sync.dma_start(
                    out=ctile[126:127, xi, :, :],
                    in_=rowv[P - NT:P, :].unsqueeze(0))
                if r < 3:
                    # p127 is never written by the row DMAs: fill with zeros
                    # once per buffer slot so exp() stays finite
                    nc.sync.dma_start(
                        out=ctile[127:128, xi, :, :],
                        in_=zpad_p.ap().rearrange("o (t c) -> o t c", c=C))
            if r == 0:
                emit_consts()
            if r == 4:
                emit_loc_dmas()

            ex = pool_e.tile([128, 2, NT, C], BF16, name="ex")
            for xi in range(2):
                nc.scalar.activation(out=ex[:, xi, :, :],
                                     in_=ctile[:, xi, :, :], func=ACT.Exp)

            # one-hot eq (broadcast operands: no DVE fast path exists)
            eq = pool_q.tile([128, NT, C], BF16, name="eq")
            if STAGE >= 3:
                nc.vector.tensor_tensor(
                    out=eq[:, :, :],
                    in0=iota81[:, :].unsqueeze(1).broadcast_to((128, NT, C)),
                    in1=ctbf[:, rc:rc + NT].unsqueeze(2).broadcast_to(
                        (128, NT, C)),
                    op=ALU.is_equal)

                # B trace: psum[m, xi, c] += sum_p eq[p,t,m] * conf[p,xi,t,c]
                for t in range(NT):
                    nc.tensor.matmul(
                        pstr2[:, :, :],
                        lhsT=eq[:, t, :],
                        rhs=ctile[:, :, t, :],
                        start=(nmm == 0), stop=(nmm == total_mm - 1))
                    nmm += 1

            # sumexp fold-tree in place over ex, both tensors per op
            if STAGE >= 4:
                f = ex[:, :, :, :]
                for w in (40, 20, 10, 5, 2, 1):
                    eng = nc.gpsimd if w in POOL_LVLS else nc.vector
                    if w == 40:
                        eng.tensor_tensor(out=f[:, :, :, 0:40],
                                          in0=f[:, :, :, 0:40],
                                          in1=f[:, :, :, 40:80], op=ALU.add)
                    else:
                        eng.tensor_tensor(out=f[:, :, :, 0:w],
                                          in0=f[:, :, :, 0:w],
                                          in1=f[:, :, :, w:2 * w], op=ALU.add)
                se = sumexp2[:, :, rc:rc + NT]
                nc.vector.tensor_tensor(out=se, in0=f[:, :, :, 0],
                                        in1=f[:, :, :, 4], op=ALU.add)
                nc.vector.tensor_tensor(out=se, in0=se,
                                        in1=f[:, :, :, 80], op=ALU.add)
                nc.vector.tensor_copy(
                    out=conf02[:, :, rc:rc + NT],
                    in_=ctile[:, :, :, 0])

            if r == 5:
                emit_loc_chain()

        # num_pos per row -> k (only needed by the tail search)
        npp = small.tile([128, 8], F32)
        nc.vector.tensor_reduce(out=npp[:, :],
                                in_=posf[:, :].rearrange("p (r t) -> p r t", r=R),
                                axis=mybir.AxisListType.X, op=ALU.add)
        ps_np = psum.tile([8, 1], F32, tag="ps")
        nc.tensor.matmul(ps_np[:, :], lhsT=npp[:, :], rhs=ones128[:, :],
                         start=True, stop=True)
        np8 = small.tile([8, 1], F32)
        nc.vector.tensor_copy(out=np8[:, :], in_=ps_np[:, :])
        k82 = small.tile([8, 2], F32)
        nc.vector.tensor_scalar(out=k82[:, 0:1], in0=np8[:, :], scalar1=3.0,
                                scalar2=float(P - 1), op0=ALU.mult, op1=ALU.min)
        nc.vector.tensor_copy(out=k82[:, 1:2], in_=k82[:, 0:1])
        nc.vector.tensor_copy(out=partials[0:8, COL_NP:COL_NP + 1],
                              in_=np8[:, :])

        # ---- tail: lse, lcm, epilogue sums ----
        if STAGE >= 5:
            # lse in place of sumexp (one Ln, one table load)
            nc.scalar.activation(out=sumexp2[:, :, :], in_=sumexp2[:, :, :],
                                 func=ACT.Ln)
            lse2 = sumexp2
            for xi in range(2):
                nc.vector.tensor_tensor(out=lcm2[xi][:, :],
                                        in0=lse2[:, xi, :],
                                        in1=conf02[:, xi, :], op=ALU.subtract)
                nc.vector.tensor_tensor(out=lcm2[xi][:, :],
                                        in0=lcm2[xi][:, :], in1=ominus[:, :],
                                        op=ALU.mult)
                # re-partition lcm into the chunk layout via a DRAM bounce:
                # chunk partition q = 16r + k holds source partitions
                # 8k..8k+7 of row r.  (A direct SBUF->SBUF partition-split
                # rearrange lowers to out-of-bounds free-dim strides.)
                nc.sync.dma_start(
                    out=scr_p.ap()[xi, :].rearrange(
                        "(r k i t) -> k i r t", r=R, k=NREP, i=8),
                    in_=lcm2[xi][:, :].rearrange(
                        "(k i) (r t) -> (k i) r t", k=NREP, r=R))
                nc.sync.dma_start(
                    out=lcmc[xi][:, :],
                    in_=scr_p.ap()[xi, :].rearrange("(q j) -> q j", q=128))

            # A = sum(lse*posf), C = sum(conf0*valid), D = sum(conf0*posf)
            # Pool mults into three distinct scratch slabs (lwd/lwu are dead
            # after the loc chain) so the DVE reduces don't serialize them.
            sj3 = (sjunk2[:, :, :],
                   lwd[:, 0:2 * RC].rearrange("p (x c) -> p x c", x=2),
                   lwu[:, 0:2 * RC].rearrange("p (x c) -> p x c", x=2))
            for src, mask, (c0, c1), sj in (
                    (lse2, posf, (COL_AT, COL_AS), sj3[0]),
                    (conf02, valid, (COL_CT, COL_CS), sj3[1]),
                    (conf02, posf, (COL_DT, COL_DS), sj3[2])):
                nc.gpsimd.tensor_tensor(
                    out=sj, in0=src[:, :, :],
                    in1=mask[:, :].unsqueeze(1).broadcast_to((128, 2, RC)),
                    op=ALU.mult)
                assert c1 == c0 + 1
                nc.vector.tensor_reduce(out=partials[:, c0:c1 + 1],
                                        in_=sj,
                                        axis=mybir.AxisListType.X, op=ALU.add)

            # B extraction from the PSUM trace
            nc.vector.tensor_tensor(
                out=sjunk2[0:81, :, 0:81], in0=pstr2[:, :, :],
                in1=eye81[:, :].unsqueeze(1).broadcast_to((81, 2, 81)),
                op=ALU.mult)
            nc.vector.tensor_reduce(out=partials[0:81, COL_BT:COL_BS + 1],
                                    in_=sjunk2[0:81, :, 0:81],
                                    axis=mybir.AxisListType.X, op=ALU.add)

        # ---- chunked binary search for per-row top-k thresholds ----
        if STAGE >= 6:
            lo82 = small.tile([8, 2], F32)
            hi82 = small.tile([8, 2], F32)
            tm82 = small.tile([8, 2], F32)
            s82 = small.tile([8, 2], F32)
            ge82 = small.tile([8, 2], I32)
            gei82 = small.tile([8, 2], I32)
            trep2 = small.tile([128, 2], F32)
            cnt2 = small.tile([128, 2], F32)
            sm2 = small.tile([128, 2], F32)
            tk82 = small.tile([8, 2], F32)

            def bcast82(vec82):
                psT = psum.tile([128, 2], F32, name="psT", tag="ps")
                nc.tensor.matmul(psT[:, :], lhsT=g8t[:, :], rhs=vec82[:, :],
                                 start=True, stop=True)
                nc.vector.tensor_copy(out=trep2[:, :], in_=psT[:, :])

            nc.gpsimd.memset(lo82[:, :], 0.0)
            nc.gpsimd.memset(hi82[:, :], HI_INIT)
            for it in range(NITER):
                nc.vector.tensor_tensor(out=tm82[:, :], in0=lo82[:, :],
                                        in1=hi82[:, :], op=ALU.add)
                nc.vector.tensor_scalar(out=tm82[:, :], in0=tm82[:, :],
                                        scalar1=0.5, scalar2=None, op0=ALU.mult)
                bcast82(tm82)
                for xi in range(2):
                    if TS_ACCUM:
                        nc.vector.tensor_scalar(
                            out=sjc[xi][:, :], in0=lcmc[xi][:, :],
                            scalar1=trep2[:, xi:xi + 1], scalar2=0.0,
                            op0=ALU.is_gt, op1=ALU.add,
                            accum_out=cnt2[:, xi:xi + 1])
                    else:
                        nc.vector.tensor_scalar(
                            out=sjc[xi][:, :], in0=lcmc[xi][:, :],
                            scalar1=trep2[:, xi:xi + 1], scalar2=None,
                            op0=ALU.is_gt)
                        nc.vector.tensor_reduce(
                            out=cnt2[:, xi:xi + 1], in_=sjc[xi][:, :],
                            axis=mybir.AxisListType.X, op=ALU.add)
                psN = psum.tile([8, 2], F32, name="psN", tag="ps")
                nc.tensor.matmul(psN[:, :], lhsT=g8[:, :], rhs=cnt2[:, :],
                                 start=True, stop=True)
                nc.vector.tensor_copy(out=s82[:, :], in_=psN[:, :])
                nc.vector.tensor_tensor(out=ge82[:, :], in0=s82[:, :],
                                        in1=k82[:, :], op=ALU.is_ge)
                nc.vector.copy_predicated(out=lo82[:, :], mask=ge82[:, :],
                                          data=tm82[:, :])
                nc.vector.tensor_scalar(out=gei82[:, :], in0=ge82[:, :],
                                        scalar1=1, scalar2=None,
                                        op0=ALU.bitwise_xor)
                nc.vector.copy_predicated(out=hi82[:, :], mask=gei82[:, :],
                                          data=tm82[:, :])

            # exact pass: topk = sum(max(v, lo)) + (k - 8832) * lo
            bcast82(lo82)
            for xi in range(2):
                if TS_ACCUM:
                    nc.vector.tensor_scalar(
                        out=sjc[xi][:, :], in0=lcmc[xi][:, :],
                        scalar1=trep2[:, xi:xi + 1], scalar2=0.0,
                        op0=ALU.max, op1=ALU.add,
                        accum_out=sm2[:, xi:xi + 1])
                else:
                    nc.vector.tensor_scalar(
                        out=sjc[xi][:, :], in0=lcmc[xi][:, :],
                        scalar1=trep2[:, xi:xi + 1], scalar2=None,
                        op0=ALU.max)
                    nc.vector.tensor_reduce(
                        out=sm2[:, xi:xi + 1], in_=sjc[xi][:, :],
                        axis=mybir.AxisListType.X, op=ALU.add)
            psE = psum.tile([8, 2], F32, name="psE", tag="ps")
            nc.tensor.matmul(psE[:, :], lhsT=g8[:, :], rhs=sm2[:, :],
                             start=True, stop=True)
            nc.vector.tensor_copy(out=tk82[:, :], in_=psE[:, :])
            nc.vector.tensor_scalar(out=s82[:, :], in0=k82[:, :],
                                    scalar1=float(NREP * NSLOT), scalar2=None,
                                    op0=ALU.subtract)
            nc.vector.tensor_tensor(out=s82[:, :], in0=s82[:, :],
                                    in1=lo82[:, :], op=ALU.mult)
            nc.vector.tensor_tensor(out=tk82[:, :], in0=tk82[:, :],
                                    in1=s82[:, :], op=ALU.add)
            nc.vector.tensor_copy(out=partials[0:8, COL_TKT:COL_TKT + 1],
                                  in_=tk82[:, 0:1])
            nc.vector.tensor_copy(out=partials[0:8, COL_TKS:COL_TKS + 1],
                                  in_=tk82[:, 1:2])

        # ---- final partition reduce of partials -> out ----
        psF = psum.tile([1, NPART], F32, name="psF", tag="ps")
        nc.tensor.matmul(psF[:, :], lhsT=ones128[:, :], rhs=partials[:, :],
                         start=True, stop=True)
        fin = small.tile([1, NPART], F32)
        nc.vector.tensor_copy(out=fin[:, :], in_=psF[:, :])
        nc.sync.dma_start(out=out_p.ap(), in_=fin[:, :])
    nc.finalize()
    return nc


_NC_CACHE = None


def _get_nc():
    global _NC_CACHE
    if _NC_CACHE is None:
        _NC_CACHE = build_nc()
    return _NC_CACHE


def _host_consts():
    eye81 = np.eye(81, dtype=ml_dtypes.bfloat16)
    g8 = np.zeros((128, 8), np.float32)
    for p in range(128):
        g8[p, p // NREP] = 1.0
    g8t = np.ascontiguousarray(g8.T)
    ones128 = np.ones((128, 1), np.float32)
    zpad = np.zeros((1, NT * C), ml_dtypes.bfloat16)
    return eye81, g8, g8t, ones128, zpad


def _ct_row_tiled(ct_rows: np.ndarray) -> np.ndarray:
    """[R, P] int -> [128, R*NT] bf16 row-tiled, pads/duplicates = -1."""
    out = np.full((128, RC), -1.0, np.float32)
    for r in range(R):
        out[0:126, r * NT:(r + 1) * NT] = ct_rows[r, 0:P_FULL].reshape(126, NT)
        out[126, r * NT + TAIL_OFF:(r + 1) * NT] = ct_rows[r, P_FULL:P]
    return out.astype(ml_dtypes.bfloat16)


def _ct_flat(ct_rows: np.ndarray) -> np.ndarray:
    flat = np.full(128 * LTT, -1.0, np.float32)
    flat[:R * P] = ct_rows.reshape(-1)
    return flat.reshape(128, LTT).astype(ml_dtypes.bfloat16)


def _build_in_maps(inputs):
    conf_T = np.asarray(inputs["conf_dataT"], np.float32)
    conf_S = np.asarray(inputs["conf_dataS"], np.float32)
    loc_T = np.asarray(inputs["loc_dataT"], np.float32)
    loc_S = np.asarray(inputs["loc_dataS"], np.float32)
    loc_t = np.asarray(inputs["loc_t"], np.float32)
    ct = np.asarray(inputs["conf_t"], np.int32)

    def _padloc(a):
        flat = a.reshape(R * P, 4)
        return np.ascontiguousarray(
            np.pad(flat, ((0, PADN), (0, 0)))).astype(ml_dtypes.bfloat16)

    eye81, g8, g8t, ones128, zpad = _host_consts()
    in_maps = []
    for d in range(NCORES):
        sl = slice(d * R, (d + 1) * R)
        ctsl = ct[sl]
        in_maps.append({
            "conf_T": np.ascontiguousarray(conf_T[sl]).astype(ml_dtypes.bfloat16),
            "conf_S": np.ascontiguousarray(conf_S[sl]).astype(ml_dtypes.bfloat16),
            "loc_T": _padloc(loc_T[sl]), "loc_S": _padloc(loc_S[sl]),
            "loc_t": _padloc(loc_t[sl]),
            "ctbf": _ct_row_tiled(ctsl),
            "ctfl": _ct_flat(ctsl),
            "eye81": eye81, "g8": g8, "g8t": g8t,
            "ones128": ones128, "zpad": zpad,
            "scr": np.zeros((2, 128 * NSLOT), ml_dtypes.bfloat16),
        })
    return in_maps


def _combine(parts):
    S = parts.astype(np.float64).sum(axis=0)
    loss_cT = S[COL_AT] - S[COL_BT] + S[COL_CT] - S[COL_DT] + S[COL_TKT]
    loss_cS = S[COL_AS] - S[COL_BS] + S[COL_CS] - S[COL_DS] + S[COL_TKS]
    N = S[COL_NP]
    return np.array([S[COL_LT] / N, loss_cT / N, S[COL_LS] / N, loss_cS / N],
                    np.float32)


def run_on_hw(inputs, trace=False, **kw):
    nc = _get_nc()
    in_maps = _build_in_maps(inputs)
    res = run_bass_kernel_spmd(nc, in_maps, core_ids=list(range(NCORES)),
                               trace=trace, **kw)
    parts = np.stack([np.asarray(r["out"]).reshape(NPART) for r in res.results])
    return _combine(parts), res


def kernel(**inputs) -> np.ndarray:
    out, _ = run_on_hw(inputs, trace=False)
    return out
